# revision 21
# baseline (speedup 1.0000x reference)
"""Trainium2 Bass kernel for nn_KPLoss_377957122199 (v2, engine-rebalanced).

loss = 1*CE + 4*smoothL1(kp) + 5*smoothL1(Procrustes rot residual)
     + 6*smoothL1(section-center diff)

Data-parallel over 8 cores (batch 8192 -> 1024/core). Key design:
  * custom DVE ops: SL1_DIFF (fused smooth-L1 sum of (in0-in1) in one
    vector op via sum f(d) = sum (d - 0.5*clamp(d))*clamp(d)), and
    ONEHOT_DOT (sum_t logits[y_t, t] via PageIdx compare, one op/chunk)
  * CE sum-of-exp on the idle TensorEngine: logits shipped twice
    (n-major [128,(n,t)] for ONEHOT; flat-transposed [100,4096] so a
    block-ones matmul reduces NS=20 on partitions into PSUM), ln reads
    PSUM packed 4 chunks/ACT via matmul tile_position.
  * keypoints host-deinterleaved to [d,k,s] bf16; kept in SBUF across
    both passes; H products/reductions and the rotation residual use
    stride-0 broadcast views (no materialized broadcasts).
  * batched 3x3 polar: 9 components contiguous [128,9*160] bf16,
    cofactors via shifted views of a 6x6-duplicated tile, scaled-Newton
    x3 + one Newton-Schulz polish, guarded (|det| clamp + Sign).
  * single ACT table set (exp/ln/sign only) - one table load.
"""

import sys
for _p in ("/opt/trn_rl_repo", "/root/.axon_site/_ro/trn_rl_repo"):
    if _p not in sys.path:
        sys.path.insert(0, _p)

import os
from contextlib import ExitStack
from operator import add as _add_op

import numpy as np
import ml_dtypes

import concourse.bass as bass
import concourse.bacc as bacc
import concourse.mybir as mybir
import concourse.tile as tile
from concourse.bass_utils import run_bass_kernel_spmd

# ---- custom DVE ops (registered at import) --------------------------------
import concourse.dve_ops as dve_ops
from concourse.dve_ops import DveOp, OPS
from concourse.dve_spec import (
    C0, C1, C2, PageIdx, Spec, Src0, Src1, Zero,
    _has_src1, eq, lower, maxx, minn, select,
)
from concourse.dve_uop import DveOpSpec


def _sl1_ref(in0, in1, s0, s1, imm2):
    d = in0.astype(np.float32) - in1.astype(np.float32)
    t = np.clip(d, s0, s1)
    return (d - imm2 * t) * t


def _oh_ref(in0, in1, s0, s1, imm2):
    raise NotImplementedError


def _register(name, spec, subdim):
    if name in dve_ops._SUB_OPCODE_FOR_NAME:
        return next(o for o in OPS if o.name == name)
    row = dve_ops._CUSTOM_DVE_ROW_BASE + len(OPS)
    assert row < 0x20
    op = DveOp(name, spec, subdim=subdim, uops_sha={})
    for ver in ("v3", "v4"):
        s = DveOpSpec(name=name, opcode=row, uops=lower(spec, ver=ver),
                      rd1_en=_has_src1(spec))
        op.uops_sha[ver] = s.sha(ver)
    OPS.append(op)
    dve_ops._SUB_OPCODE_FOR_NAME[name] = row
    return op


_d = Src0 - Src1
_t = minn(maxx(_d, C0), C1)
SL1_DIFF = _register("SL1_DIFF", Spec(body=(_d - _t * C2) * _t, accum=_add_op,
                                      reference=_sl1_ref), subdim=False)
_pg = PageIdx(C0, C1)
ONEHOT_DOT = _register("ONEHOT_DOT",
                       Spec(body=select(eq(Src1, _pg), Src0, Zero),
                            accum=_add_op, reference=_oh_ref), subdim=True)

FP32 = mybir.dt.float32
BF16 = mybir.dt.bfloat16
AX = mybir.AxisListType
OP = mybir.AluOpType
AF = mybir.ActivationFunctionType

N_CORES = 8
B, K, NS, SEC = 8192, 400, 20, 20
S = K // SEC                    # 20 sections / sample
BC = B // N_CORES               # 1024 samples / core
NCH_KP = BC // 128              # 8 keypoint chunks
SFD = NCH_KP * S                # 160 sections per partition
NCH_CE = 20                     # CE chunks
TOKC = BC * K // NCH_CE         # 20480 tokens / CE chunk
T_CE = TOKC // 128              # 160 tokens / partition (n-major layout)
FFL = TOKC * NS // 100          # 4096 cols in flat [100, .] layout

N_ITER = 2                      # polar Newton iterations

# acc column map
C_LSE = 0                       # 5 cols (groups of 4 chunks; rows 32q+0..4)
C_LY = C_LSE + 5                # 20 cols
C_KP = C_LY + NCH_CE            # 8
C_ROT = C_KP + NCH_KP           # 8
C_CENT = C_ROT + NCH_KP         # 1
NACC = C_CENT + 1


def _emit(ctx, tc, aps):
    nc = tc.nc
    pk, gk, lgn, lgf, lb, ob, out = (aps[k] for k in
                                     ("pk", "gk", "lgn", "lgf", "lb", "ob", "out"))

    pers = ctx.enter_context(tc.tile_pool(name="pers", bufs=1))
    scr = ctx.enter_context(tc.tile_pool(name="scr", bufs=1))
    cep = ctx.enter_context(tc.tile_pool(name="ce", bufs=3))
    psp = ctx.enter_context(tc.tile_pool(name="ps", bufs=1, space="PSUM"))

    acc = pers.tile([128, NACC], FP32, tag="acc", name="acc")
    oneblk = pers.tile([100, 5], BF16, tag="oneblk", name="oneblk")
    nc.sync.dma_start(oneblk[:], ob)
    lnhalf = pers.tile([128, 1], FP32, tag="lnhalf", name="lnhalf")
    nc.gpsimd.memset(lnhalf[:], float(np.log(0.5)))

    # keypoint chunks persist across phase 1 and 3
    pb = [pers.tile([128, 1200], BF16, tag=f"pb{c}", name=f"pb{c}") for c in range(NCH_KP)]
    gb = [pers.tile([128, 1200], BF16, tag=f"gb{c}", name=f"gb{c}") for c in range(NCH_KP)]

    # ---------------- cross entropy ----------------
    psum = psp.tile([128, FFL], FP32, tag="mm", name="mm")

    def ce_chunk(c):
        lgnc = cep.tile([128, NS * T_CE], BF16, tag="lgn", name="lgn")
        nc.sync.dma_start(lgnc[:], lgn[c])
        lbc = cep.tile([128, T_CE], BF16, tag="lbc", name="lbc")
        nc.sync.dma_start(lbc[:], lb[c])
        lgfc = cep.tile([100, FFL], BF16, tag="lgf", name="lgf")
        nc.sync.dma_start(lgfc[:], lgf[c])

        # l_y: one custom op
        dmp = scr.tile([128, NS * T_CE], BF16, tag="dmp", name="dmp")
        nc.vector._custom_dve(
            ONEHOT_DOT,
            out=dmp[:].rearrange("p (n t) -> p n t", n=NS),
            in0=lgnc[:].rearrange("p (n t) -> p n t", n=NS),
            in1=lbc[:].unsqueeze(1).broadcast_to([128, NS, T_CE]),
            s0=0.0, s1=1.0, accum_out=acc[:, C_LY + c:C_LY + c + 1])

        # lse: exp (scalar) -> block-ones matmul (PE) -> ln on packed PSUM
        ex = lgfc
        nc.scalar.activation(ex[:], lgfc[:], AF.Exp)
        q = c % 4
        for h in range(FFL // 512):
            nc.tensor.matmul(
                psum[32 * q:32 * q + 5, h * 512:(h + 1) * 512],
                oneblk[:], ex[:, h * 512:(h + 1) * 512],
                start=True, stop=True, tile_position=(0, 32 * q))
        if q == 3:
            g = c // 4
            lnd = scr.tile([101, FFL], BF16, tag="lnd", name="lnd")
            nc.scalar.activation(lnd[:], psum[0:101, :], AF.Ln,
                                 accum_out=acc[0:101, C_LSE + g:C_LSE + g + 1])

    for c in range(2):
        ce_chunk(c)
    for c in range(NCH_KP):
        nc.sync.dma_start(pb[c][:], pk[c])
        nc.sync.dma_start(gb[c][:], gk[c])
    for c in range(2, 12):
        ce_chunk(c)

    # ---------------- phase 1: keypoints ----------------
    H = pers.tile([128, 9 * SFD], BF16, tag="H", name="H")
    sp = pers.tile([128, 3 * SFD], BF16, tag="sp", name="sp")
    sg = pers.tile([128, 3 * SFD], BF16, tag="sg", name="sg")

    wk1 = tc.alloc_tile_pool(name="wk1", bufs=2)
    work = wk1
    for c in range(NCH_KP):
        p3 = pb[c][:].rearrange("p (d f) -> p d f", d=3)        # [128,3,400]
        g3 = gb[c][:].rearrange("p (d f) -> p d f", d=3)
        dmp = scr.tile([128, 1200], BF16, tag="dump1200", name="dump1200")
        nc.vector._custom_dve(SL1_DIFF, out=dmp[:], in0=pb[c][:], in1=gb[c][:],
                              s0=-1.0, s1=1.0, imm2=0.5,
                              accum_out=acc[:, C_KP + c:C_KP + c + 1])
        # H products: T[m=(i,j)] = g_i * p_j over (k,s)
        T = work.tile([128, 9 * 400], BF16, tag="hT", name="hT")
        nc.vector.tensor_tensor(
            T[:].rearrange("p (i j f) -> p i j f", i=3, j=3),
            g3.unsqueeze(2).broadcast_to([128, 3, 3, 400]),
            p3.unsqueeze(1).broadcast_to([128, 3, 3, 400]), OP.mult)
        # k-tree: 20 -> 10 -> 5 -> reduce
        T4 = T[:].rearrange("p (m k s) -> p m k s", m=9, k=SEC)
        A1 = work.tile([128, 9 * 10 * S], BF16, tag="hA1", name="hA1")
        A1v = A1[:].rearrange("p (m k s) -> p m k s", m=9, k=10, s=S)
        nc.vector.tensor_tensor(A1v, T4[:, :, 0:10], T4[:, :, 10:20], OP.add)
        A2 = work.tile([128, 9 * 5 * S], BF16, tag="hA2", name="hA2")
        A2v = A2[:].rearrange("p (m k s) -> p m k s", m=9, k=5, s=S)
        nc.vector.tensor_tensor(A2v, A1v[:, :, 0:5], A1v[:, :, 5:10], OP.add)
        D1 = work.tile([128, 9 * 2 * S], BF16, tag="hD1", name="hD1")
        D1v = D1[:].rearrange("p (m k s) -> p m k s", m=9, k=2, s=S)
        nc.vector.tensor_tensor(D1v, A2v[:, :, 0:2], A2v[:, :, 2:4], OP.add)
        D2 = work.tile([128, 9 * S], BF16, tag="hD2", name="hD2")
        D2v = D2[:].rearrange("p (m s) -> p m s", m=9)
        nc.vector.tensor_tensor(D2v, D1v[:, :, 0], D1v[:, :, 1], OP.add)
        nc.vector.tensor_tensor(
            H[:].rearrange("p (m f) -> p m f", m=9)[:, :, c * S:(c + 1) * S],
            D2v, A2v[:, :, 4], OP.add)
        # point sums over k (tree adds, all 2x-mode)
        for src, dst in ((p3, sp), (g3, sg)):
            s4 = src.rearrange("p d (k s) -> p d k s", k=SEC)
            B1 = work.tile([128, 3 * 10 * S], BF16, tag="sB1", name="sB1")
            B1v = B1[:].rearrange("p (d k s) -> p d k s", d=3, k=10, s=S)
            nc.vector.tensor_tensor(B1v, s4[:, :, 0:10], s4[:, :, 10:20], OP.add)
            B2 = work.tile([128, 3 * 5 * S], BF16, tag="sB2", name="sB2")
            B2v = B2[:].rearrange("p (d k s) -> p d k s", d=3, k=5, s=S)
            nc.vector.tensor_tensor(B2v, B1v[:, :, 0:5], B1v[:, :, 5:10], OP.add)
            C1t = work.tile([128, 3 * 2 * S], BF16, tag="sC1", name="sC1")
            C1v = C1t[:].rearrange("p (d k s) -> p d k s", d=3, k=2, s=S)
            nc.vector.tensor_tensor(C1v, B2v[:, :, 0:2], B2v[:, :, 2:4], OP.add)
            C2t = work.tile([128, 3 * S], BF16, tag="sC2", name="sC2")
            C2v = C2t[:].rearrange("p (d s) -> p d s", d=3)
            nc.vector.tensor_tensor(C2v, C1v[:, :, 0], C1v[:, :, 1], OP.add)
            nc.vector.tensor_tensor(
                dst[:].rearrange("p (d f) -> p d f", d=3)[:, :, c * S:(c + 1) * S],
                C2v, B2v[:, :, 4], OP.add)

    wk1.release()
    # center loss: smoothL1((sp-sg)/SEC) over [128, 3*SFD]
    sps = pers.tile([128, 3 * SFD], BF16, tag="sps", name="sps")
    sgs = pers.tile([128, 3 * SFD], BF16, tag="sgs", name="sgs")
    nc.vector.tensor_scalar(sps[:], sp[:], 1.0 / SEC, None, OP.mult)
    nc.vector.tensor_scalar(sgs[:], sg[:], 1.0 / SEC, None, OP.mult)
    dmpc = scr.tile([128, 3 * SFD], BF16, tag="dmpc", name="dmpc")
    nc.vector._custom_dve(SL1_DIFF, out=dmpc[:], in0=sps[:], in1=sgs[:],
                          s0=-1.0, s1=1.0, imm2=0.5,
                          accum_out=acc[:, C_CENT:C_CENT + 1])

    # H -= sg_i * sp_j / SEC
    sp3 = sp[:].rearrange("p (d f) -> p d f", d=3)
    sg3 = sg[:].rearrange("p (d f) -> p d f", d=3)
    M = scr.tile([128, 9 * SFD], BF16, tag="hcM", name="hcM")
    nc.vector.tensor_tensor(
        M[:].rearrange("p (i j f) -> p i j f", i=3, j=3),
        sg3.unsqueeze(2).broadcast_to([128, 3, 3, SFD]),
        sp3.unsqueeze(1).broadcast_to([128, 3, 3, SFD]), OP.mult)
    nc.vector.tensor_scalar(M[:], M[:], 1.0 / SEC, None, OP.mult)
    nc.vector.tensor_tensor(H[:], H[:], M[:], OP.subtract)

    # ---------------- polar decomposition (batched 3x3, bf16) ----------------
    pol = tc.alloc_tile_pool(name="pol", bufs=1)
    A66 = pol.tile([128, 36 * SFD], BF16, tag="A66", name="A66")
    A = A66[:].rearrange("p (a b f) -> p a b f", a=6, b=6)
    X = A[:, 0:3, 0:3]                                  # X lives inside A66
    H4 = H[:].rearrange("p (i j f) -> p i j f", i=3, j=3)
    nc.vector.tensor_copy(X, H4)
    Cf = pol.tile([128, 9 * SFD], BF16, tag="cof", name="cof")
    C3v = Cf[:].rearrange("p (i j f) -> p i j f", i=3, j=3)
    SX = pol.tile([128, 9 * SFD], BF16, tag="sqX", name="sqX")
    det = pol.tile([128, SFD], BF16, tag="det", name="det")
    t160a = pol.tile([128, SFD], FP32, tag="t160a", name="t160a")
    t160b = pol.tile([128, SFD], FP32, tag="t160b", name="t160b")
    adet = pol.tile([128, SFD], FP32, tag="adet", name="adet")
    sgn = pol.tile([128, SFD], BF16, tag="sgn", name="sgn")
    nx2 = pol.tile([128, SFD], BF16, tag="nx2", name="nx2")
    nc2_ = pol.tile([128, SFD], BF16, tag="nc2", name="nc2")
    zln = pol.tile([128, SFD], FP32, tag="zln", name="zln")
    hz = pol.tile([128, SFD], BF16, tag="hz", name="hz")
    wz = pol.tile([128, SFD], BF16, tag="wz", name="wz")
    wf = pol.tile([128, SFD], FP32, tag="wf", name="wf")

    def frob(dst, src4):
        sxw = SX[:].rearrange("p (i j f) -> p i j f", i=3, j=3)
        nc.vector.tensor_tensor(sxw, src4, src4, OP.mult)
        sx = SX[:].rearrange("p (m f) -> p m f", m=9)
        q1 = pol.tile([128, 4 * SFD], BF16, tag="fq1", name="fq1")
        q1v = q1[:].rearrange("p (m f) -> p m f", m=4)
        nc.vector.tensor_tensor(q1v, sx[:, 0:4], sx[:, 4:8], OP.add)
        q2 = pol.tile([128, 2 * SFD], BF16, tag="fq2", name="fq2")
        q2v = q2[:].rearrange("p (m f) -> p m f", m=2)
        nc.vector.tensor_tensor(q2v, q1v[:, 0:2], q1v[:, 2:4], OP.add)
        nc.vector.tensor_tensor(dst.unsqueeze(1), q2v[:, 0:1], q2v[:, 1:2], OP.add)
        nc.vector.tensor_tensor(dst, dst, sx[:, 8], OP.add)

    for it in range(N_ITER):
        # duplicate X -> A66 quadrants
        nc.vector.tensor_copy(A[:, 0:3, 3:6], X)
        nc.vector.tensor_copy(A[:, 3:6, :], A[:, 0:3, :])
        # cofactors: C[i][j] = A[i+1][j+1]A[i+2][j+2] - A[i+1][j+2]A[i+2][j+1]
        T1 = pol.tile([128, 9 * SFD], BF16, tag="ct1", name="ct1")
        nc.vector.tensor_tensor(
            T1[:].rearrange("p (i j f) -> p i j f", i=3, j=3),
            A[:, 1:4, 1:4], A[:, 2:5, 2:5], OP.mult)
        T2 = pol.tile([128, 9 * SFD], BF16, tag="ct2", name="ct2")
        nc.vector.tensor_tensor(
            T2[:].rearrange("p (i j f) -> p i j f", i=3, j=3),
            A[:, 1:4, 2:5], A[:, 2:5, 1:4], OP.mult)
        nc.vector.tensor_tensor(Cf[:], T1[:], T2[:], OP.subtract)
        # det = sum_j X[0][j] * C[0][j]
        P0 = pol.tile([128, 3 * SFD], BF16, tag="dp0", name="dp0")
        P0v = P0[:].rearrange("p (j f) -> p j f", j=3)
        nc.vector.tensor_tensor(P0v, X[:, 0], C3v[:, 0], OP.mult)
        nc.vector.tensor_tensor(det[:].unsqueeze(1), P0v[:, 0:1], P0v[:, 1:2], OP.add)
        nc.vector.tensor_tensor(det[:], det[:], P0v[:, 2], OP.add)
        # guards + zeta = exp(0.25 ln(nC2/nX2) - 0.5 ln|det|)
        frob(nx2[:], X)
        frob(nc2_[:], C3v)
        nc.vector.tensor_scalar(nx2[:], nx2[:], 1e-12, None, OP.max)
        nc.vector.tensor_scalar(nc2_[:], nc2_[:], 1e-12, None, OP.max)
        nc.vector.tensor_scalar(sgn[:], det[:], 0.0, None, OP.is_ge)
        nc.vector.tensor_scalar(sgn[:], sgn[:], 2.0, -1.0, OP.mult, OP.add)
        nc.vector.tensor_tensor(adet[:], det[:], sgn[:], OP.mult)
        nc.vector.tensor_scalar(adet[:], adet[:], 1e-6, None, OP.max)
        # u = nC2 / (nX2 * adet^2); zln = 0.25 ln u  (one Ln, one Exp)
        nc.vector.tensor_copy(t160a[:], nx2[:])
        nc.vector.reciprocal_approx_fast(t160b[:], t160a[:])
        nc.vector.reciprocal_approx_fast(wf[:], adet[:])
        nc.vector.tensor_tensor(t160a[:], wf[:], wf[:], OP.mult)
        nc.vector.tensor_tensor(t160b[:], t160b[:], t160a[:], OP.mult)
        nc.vector.tensor_tensor(t160b[:], t160b[:], nc2_[:], OP.mult)
        nc.scalar.activation(zln[:], t160b[:], AF.Ln)
        nc.vector.tensor_scalar(zln[:], zln[:], 0.25, None, OP.mult)
        nc.scalar.activation(t160a[:], zln[:], AF.Exp)
        nc.vector.tensor_scalar(hz[:], t160a[:], 0.5, None, OP.mult)
        nc.vector.tensor_tensor(t160b[:], t160a[:], adet[:], OP.mult)
        nc.vector.tensor_scalar(t160b[:], t160b[:], 2.0, None, OP.mult)
        nc.vector.reciprocal_approx_fast(wf[:], t160b[:])
        nc.vector.tensor_tensor(wz[:], wf[:], sgn[:], OP.mult)
        # X = X*hz + C*w  (broadcast over 9 components)
        hzb = hz[:].unsqueeze(1).unsqueeze(1).broadcast_to([128, 3, 3, SFD])
        wzb = wz[:].unsqueeze(1).unsqueeze(1).broadcast_to([128, 3, 3, SFD])
        U1 = pol.tile([128, 9 * SFD], BF16, tag="u1", name="u1")
        U1v = U1[:].rearrange("p (i j f) -> p i j f", i=3, j=3)
        nc.vector.tensor_tensor(U1v, X, hzb, OP.mult)
        U2 = pol.tile([128, 9 * SFD], BF16, tag="u2", name="u2")
        U2v = U2[:].rearrange("p (i j f) -> p i j f", i=3, j=3)
        nc.vector.tensor_tensor(U2v, C3v, wzb, OP.mult)
        nc.vector.tensor_tensor(X, U1v, U2v, OP.add)

    # Newton-Schulz polish: R = X (1.5 I - 0.5 X^T X)
    Y = pol.tile([128, 9 * SFD], BF16, tag="Y", name="Y")
    Yv = Y[:].rearrange("p (i j f) -> p i j f", i=3, j=3)
    Tk = pol.tile([128, 9 * SFD], BF16, tag="Tk", name="Tk")
    Tkv = Tk[:].rearrange("p (i j f) -> p i j f", i=3, j=3)
    for k in range(3):
        xk = A[:, k, 0:3]                               # [128, 3, SFD] = X[k][*]
        dst = Yv if k == 0 else Tkv
        nc.vector.tensor_tensor(
            dst, xk.unsqueeze(2).broadcast_to([128, 3, 3, SFD]),
            xk.unsqueeze(1).broadcast_to([128, 3, 3, SFD]), OP.mult)
        if k:
            nc.vector.tensor_tensor(Y[:], Y[:], Tk[:], OP.add)
    W = pol.tile([128, 9 * SFD], BF16, tag="W", name="W")
    nc.vector.tensor_scalar(W[:], Y[:], -0.5, None, OP.mult)
    Wv = W[:].rearrange("p (m f) -> p m f", m=9)
    for m in (0, 4, 8):
        nc.vector.tensor_scalar(Wv[:, m], Wv[:, m], 1.5, None, OP.add)
    R = pers.tile([128, 9 * SFD], BF16, tag="R", name="R")
    Rv = R[:].rearrange("p (i j f) -> p i j f", i=3, j=3)
    Wv4 = W[:].rearrange("p (k j f) -> p k j f", k=3, j=3)
    for k in range(3):
        dst = Rv if k == 0 else Tkv
        nc.vector.tensor_tensor(
            dst, A[:, 0:3, k].unsqueeze(2).broadcast_to([128, 3, 3, SFD]),
            Wv4[:, k].unsqueeze(1).broadcast_to([128, 3, 3, SFD]), OP.mult)
        if k:
            nc.vector.tensor_tensor(R[:], R[:], Tk[:], OP.add)

    # v_j = (sum_i sp_i R_ij - sg_j)/SEC
    v = pers.tile([128, 3 * SFD], BF16, tag="v", name="v")
    vv = v[:].rearrange("p (j f) -> p j f", j=3)
    Pv = pol.tile([128, 9 * SFD], BF16, tag="Pv", name="Pv")
    Pvv = Pv[:].rearrange("p (i j f) -> p i j f", i=3, j=3)
    nc.vector.tensor_tensor(
        Pvv, sp3.unsqueeze(2).broadcast_to([128, 3, 3, SFD]), Rv, OP.mult)
    nc.vector.tensor_tensor(vv, Pvv[:, 0], Pvv[:, 1], OP.add)
    nc.vector.tensor_tensor(vv, vv, Pvv[:, 2], OP.add)
    nc.vector.tensor_tensor(vv, vv, sg3, OP.subtract)
    nc.vector.tensor_scalar(v[:], v[:], 1.0 / SEC, None, OP.mult)

    pol.release()
    # ---------------- phase 3: rotation residual ----------------
    wk3 = tc.alloc_tile_pool(name="wk3", bufs=2)
    work = wk3
    for c in range(NCH_KP):
        p3 = pb[c][:].rearrange("p (d k s) -> p d k s", d=3, k=SEC)
        g3 = gb[c][:].rearrange("p (d k s) -> p d k s", d=3, k=SEC)
        Rc = Rv[:, :, :, c * S:(c + 1) * S]              # [128,3,3,S]
        vc = vv[:, :, c * S:(c + 1) * S]                 # [128,3,S]
        T0 = work.tile([128, 1200], BF16, tag="r0", name="r0")
        T1_ = work.tile([128, 1200], BF16, tag="r1", name="r1")
        T2_ = work.tile([128, 1200], BF16, tag="r2", name="r2")
        for i, Td in enumerate((T0, T1_, T2_)):
            nc.vector.tensor_tensor(
                Td[:].rearrange("p (j k s) -> p j k s", j=3, k=SEC),
                p3[:, i].unsqueeze(1).broadcast_to([128, 3, SEC, S]),
                Rc[:, i].unsqueeze(2).broadcast_to([128, 3, SEC, S]), OP.mult)
        nc.gpsimd.tensor_tensor(T0[:], T0[:], T1_[:], OP.add)
        Bt = work.tile([128, 1200], BF16, tag="rB", name="rB")
        nc.gpsimd.tensor_tensor(
            Bt[:].rearrange("p (j k s) -> p j k s", j=3, k=SEC),
            g3, vc.unsqueeze(2).broadcast_to([128, 3, SEC, S]), OP.add)
        nc.vector.tensor_tensor(Bt[:], Bt[:], T2_[:], OP.subtract)
        dmp = scr.tile([128, 1200], BF16, tag="dump1200", name="dump1200")
        nc.vector._custom_dve(SL1_DIFF, out=dmp[:], in0=T0[:], in1=Bt[:],
                              s0=-1.0, s1=1.0, imm2=0.5,
                              accum_out=acc[:, C_ROT + c:C_ROT + c + 1])

    wk3.release()
    for c in range(12, NCH_CE):
        ce_chunk(c)
    nc.sync.dma_start(out[:], acc[:])


_CACHE = {}


def _build():
    if "nc" in _CACHE:
        return _CACHE["nc"]
    nc = bacc.Bacc("TRN2", target_bir_lowering=False, debug=False,
                   enable_asserts=False, num_devices=N_CORES)
    aps = {
        "pk": nc.dram_tensor("pk", [NCH_KP, 128, 1200], BF16, kind="ExternalInput").ap(),
        "gk": nc.dram_tensor("gk", [NCH_KP, 128, 1200], BF16, kind="ExternalInput").ap(),
        "lgn": nc.dram_tensor("lgn", [NCH_CE, 128, NS * T_CE], BF16, kind="ExternalInput").ap(),
        "lgf": nc.dram_tensor("lgf", [NCH_CE, 100, FFL], BF16, kind="ExternalInput").ap(),
        "lb": nc.dram_tensor("lb", [NCH_CE, 128, T_CE], BF16, kind="ExternalInput").ap(),
        "ob": nc.dram_tensor("ob", [100, 5], BF16, kind="ExternalInput").ap(),
        "out": nc.dram_tensor("out", [128, NACC], FP32, kind="ExternalOutput").ap(),
    }
    with tile.TileContext(nc) as tc:
        with ExitStack() as ctx:
            with nc.allow_low_precision(reason="bf16 5-term tree reduces; validated"):
                _emit(ctx, tc, aps)
    nc.compile()
    _CACHE["nc"] = nc
    return nc


def _shard_inputs(pred_keypoints, gt_keypoints, pred_section_logits, gt_section_label):
    bf = ml_dtypes.bfloat16
    pkh = np.asarray(pred_keypoints, dtype=np.float32).reshape(
        N_CORES, NCH_KP, 128, S, SEC, 3).transpose(0, 1, 2, 5, 4, 3)
    pkh = np.ascontiguousarray(pkh).reshape(N_CORES, NCH_KP, 128, 1200).astype(bf)
    gkh = np.asarray(gt_keypoints, dtype=np.float32).reshape(
        N_CORES, NCH_KP, 128, S, SEC, 3).transpose(0, 1, 2, 5, 4, 3)
    gkh = np.ascontiguousarray(gkh).reshape(N_CORES, NCH_KP, 128, 1200).astype(bf)
    lg32 = np.asarray(pred_section_logits, dtype=np.float32)
    lgnh = lg32.reshape(N_CORES, NCH_CE, 128, T_CE, NS).transpose(0, 1, 2, 4, 3)
    lgnh = np.ascontiguousarray(lgnh).reshape(N_CORES, NCH_CE, 128, NS * T_CE).astype(bf)
    lgfh = lg32.reshape(N_CORES, NCH_CE, FFL, 100).transpose(0, 1, 3, 2)
    lgfh = np.ascontiguousarray(lgfh).astype(bf)
    lbh = np.asarray(gt_section_label).reshape(N_CORES, NCH_CE, 128, T_CE).astype(bf)
    ob = np.zeros((100, 5), dtype=np.float32)
    for g in range(5):
        ob[g * 20:(g + 1) * 20, g] = 1.0
    ob = ob.astype(bf)
    return [{"pk": pkh[i], "gk": gkh[i], "lgn": lgnh[i], "lgf": lgfh[i],
             "lb": lbh[i], "ob": ob} for i in range(N_CORES)]


def combine_accs(accs):
    tot = np.zeros(NACC, dtype=np.float64)
    lse = 0.0
    for a in accs:
        a64 = a.astype(np.float64)
        tot += a64.sum(axis=0)
        for g in range(5):
            col = a64[:, C_LSE + g]
            for q in range(4):
                lse += col[32 * q:32 * q + 5].sum()
    ly = tot[C_LY:C_LY + NCH_CE].sum()
    kp = tot[C_KP:C_KP + NCH_KP].sum()
    rot = tot[C_ROT:C_ROT + NCH_KP].sum()
    cent = tot[C_CENT]
    total = (1.0 * (lse - ly) / (B * K)
             + 4.0 * kp / (B * K * 3)
             + 5.0 * rot / (B * K * 3)
             + 6.0 * cent / (B * S * 3))
    return np.float32(total)


def kernel(**inputs) -> np.ndarray:
    nc = _build()
    in_maps = _shard_inputs(**inputs)
    res = run_bass_kernel_spmd(nc, in_maps, list(range(N_CORES))).results
    return combine_accs([res[i]["out"] for i in range(N_CORES)])
